# revision 17
# baseline (speedup 1.0000x reference)
"""AttnPooling Trainium2 Bass kernel (8-core SPMD).

Math (per graph g, head k):
  scores = tanh(h@W1+b1)@W2+b2                  [N, 8]
  e      = exp(scores)            (no max-sub; scores are O(5), safe)
  s_gk   = sum_{i in g} e_ik * h_i              [G, 8, 256]
  d_gk   = sum_{i in g} e_ik                    [G, 8]
  out_g  = (1/8) sum_k s_gk / d_gk              [G, 256]

Sharding: graphs are contiguous under sorted segment_ids; each of the 8
cores takes a contiguous block of G/8 graphs.  Inside a core, graphs are
grouped into windows of 16 (16 graphs x 8 heads = 128 PSUM partitions);
each window's node count is padded to a multiple of 128 so every tile
belongs to exactly one window and the program structure is identical
across cores (only the data differs -> single SPMD program).

Per 128-node tile:
  fc1:  t1[128d, F] += W1c.T @ hT_c          (bf16, hT shipped from host)
  tanh: a1 = tanh(t1 + b1)                   (ACT, bf16 out)
  fc2:  sco[128n, 8] = a1_slice.T @ W2       (a1 is the stationary)
  exp:  e = exp(sco + b2)                    (ACT, node-major bf16)
  E[i, g*8+k] = M[i,g] * e[i,k]              (DVE broadcast multiply)
  psumW[128, 257] += E.T @ hb[:, 0:257]      (col 256 of hb is 1.0 -> denom)
Window drain:
  rc = 1/max(denom, eps); ssc = psumW[:, :256]*rc (bf16)
  outp[16, 256] = S16.T @ ssc   (S16[(g,k),g] = 1/8)  -> DMA to HBM
"""

import os
import numpy as np
import ml_dtypes

BF16 = ml_dtypes.bfloat16

N_CORES = 8
WSIZE = 16          # graphs per window (16*8 heads = 128 partitions)
H = 8               # heads
F_IN = 256          # in_features
D = 128             # dense dim
HB_W = 258          # 256 feat + 1 ones + 1 gidx col
MACRO = 8           # slots per macro

_PROGRAM_CACHE = {}


# ----------------------------------------------------------------- host prep
def _preprocess(h, segment_ids, num_graphs):
    N = h.shape[0]
    G = int(num_graphs)
    counts = np.bincount(segment_ids, minlength=G).astype(np.int64)
    g_core = -(-G // N_CORES)
    n_win = -(-g_core // WSIZE)
    starts = np.zeros(G + 1, dtype=np.int64)
    np.cumsum(counts, out=starts[1:])

    # LPT-balance graphs into N_CORES*n_win bins of exactly WSIZE graphs
    n_bins = N_CORES * n_win
    import heapq
    heap = [(0, b, 0) for b in range(n_bins)]   # (load, bin, count)
    heapq.heapify(heap)
    bins = [[] for _ in range(n_bins)]
    for g in np.argsort(-counts, kind="stable"):
        while True:
            load, b, cnt = heapq.heappop(heap)
            if cnt < WSIZE:
                break
        bins[b].append(int(g))
        heapq.heappush(heap, (load + int(counts[g]), b, cnt + 1))
    bin_nodes = np.array([sum(counts[g] for g in bb) for bb in bins])
    t_w = int(max(1, -(-bin_nodes.max() // 128)))
    npad = n_win * t_w * 128
    B = t_w * 128

    h32 = np.ascontiguousarray(h, dtype=np.float32)
    hb_all, ht0_all, ht1_all = [], [], []
    row2graph = np.full((N_CORES, n_win * WSIZE), -1, dtype=np.int64)
    for c in range(N_CORES):
        hb = np.zeros((npad, HB_W), dtype=BF16)
        hb[:, F_IN] = 1.0
        hb[:, F_IN + 1] = 255.0          # pad rows match no window graph
        hpad = np.zeros((npad, F_IN), dtype=np.float32)
        for w in range(n_win):
            bb = bins[c * n_win + w]
            r = w * B
            for idx, g in enumerate(bb):
                row2graph[c, w * WSIZE + idx] = g
                n0, n1 = starts[g], starts[g + 1]
                nw = n1 - n0
                if nw == 0:
                    continue
                hpad[r:r + nw] = h32[n0:n1]
                hb[r:r + nw, :F_IN] = h32[n0:n1].astype(BF16)
                hb[r:r + nw, F_IN + 1] = float(idx)
                r += nw
        hT = hpad.T.astype(BF16)
        hT = hT.reshape(F_IN, n_win, 128, t_w)
        hT = np.ascontiguousarray(
            hT.transpose(0, 1, 3, 2)).reshape(F_IN, npad)
        hb_all.append(hb)
        ht0_all.append(np.ascontiguousarray(hT[:D]))
        ht1_all.append(np.ascontiguousarray(hT[D:]))
    meta = dict(G=G, g_core=g_core, n_win=n_win, t_w=t_w, npad=npad,
                row2graph=row2graph)
    return hb_all, ht0_all, ht1_all, meta


def _const_inputs(W1, b1, W2, b2):
    W1 = np.asarray(W1, dtype=np.float32)
    W2 = np.asarray(W2, dtype=np.float32)
    s16 = np.zeros((WSIZE * H, WSIZE), dtype=BF16)
    for g in range(WSIZE):
        s16[g * H:(g + 1) * H, g] = 0.125
    return {
        "w1c0": np.ascontiguousarray(W1[:D].astype(BF16)),       # [128,128]
        "w1c1": np.ascontiguousarray(W1[D:].astype(BF16)),       # [128,128]
        "w2": np.ascontiguousarray(W2.astype(BF16)),             # [128,8]
        "b1": np.asarray(b1, dtype=np.float32).reshape(D, 1),
        "s16": s16,                                              # [128,16]
        "iota16": np.tile(np.arange(WSIZE, dtype=BF16), (WSIZE * H, 1)),
    }


# ------------------------------------------------------------- device program
def _build_program(n_win, t_w, npad, num_devices, reps=1):
    import concourse.bacc as bacc
    import concourse.mybir as mybir
    from concourse import tile

    dt = mybir.dt
    AF = mybir.ActivationFunctionType
    B = t_w * 128

    nc = bacc.Bacc("TRN2", target_bir_lowering=False, debug=False,
                   enable_asserts=False, num_devices=num_devices)

    hb_d = nc.dram_tensor("hb", [npad, HB_W], dt.bfloat16,
                          kind="ExternalInput")
    ht0_d = nc.dram_tensor("ht0", [D, npad], dt.bfloat16,
                           kind="ExternalInput")
    ht1_d = nc.dram_tensor("ht1", [D, npad], dt.bfloat16,
                           kind="ExternalInput")
    w1c0_d = nc.dram_tensor("w1c0", [D, D], dt.bfloat16,
                            kind="ExternalInput")
    w1c1_d = nc.dram_tensor("w1c1", [D, D], dt.bfloat16,
                            kind="ExternalInput")
    w2_d = nc.dram_tensor("w2", [D, H], dt.bfloat16, kind="ExternalInput")
    b1_d = nc.dram_tensor("b1", [D, 1], dt.float32, kind="ExternalInput")
    s16_d = nc.dram_tensor("s16", [WSIZE * H, WSIZE], dt.bfloat16,
                           kind="ExternalInput")
    iota16_d = nc.dram_tensor("iota16", [WSIZE * H, WSIZE], dt.bfloat16,
                              kind="ExternalInput")
    out_d = nc.dram_tensor("out", [n_win * WSIZE, F_IN], dt.float32,
                           kind="ExternalOutput")

    # window-blocked view: [w, p, (t f)]
    hb_wv = hb_d.ap().rearrange("(w p t) f -> w p (t f)", p=128, t=t_w)

    # macro slot ranges
    macros = []
    j0 = 0
    while j0 < t_w:
        macros.append((j0, min(MACRO, t_w - j0)))
        j0 += macros[-1][1]

    import contextlib
    with tile.TileContext(nc) as tc:
        with (
            tc.tile_pool(name="consts", bufs=1) as cpool,
            tc.tile_pool(name="hbp", bufs=4) as hbp,
            tc.tile_pool(name="htp", bufs=4) as htp,
            tc.tile_pool(name="actp", bufs=3) as actp,
            tc.tile_pool(name="ep", bufs=3) as epool,
            tc.tile_pool(name="drainp", bufs=2) as drainp,
            tc.tile_pool(name="ps_mm", bufs=3, space="PSUM") as ps_mm,
            tc.tile_pool(name="ps_sco", bufs=2, space="PSUM") as ps_sco,
            tc.tile_pool(name="ps_w", bufs=2, space="PSUM") as ps_w,
            tc.tile_pool(name="ps_out", bufs=1, space="PSUM") as ps_out,
        ):
            w1c0 = cpool.tile([D, D], dt.bfloat16)
            w1c1 = cpool.tile([D, D], dt.bfloat16)
            w2 = cpool.tile([D, H], dt.bfloat16)
            b1 = cpool.tile([D, 1], dt.float32)
            s16 = cpool.tile([WSIZE * H, WSIZE], dt.bfloat16)
            iota16 = cpool.tile([WSIZE * H, WSIZE], dt.bfloat16)
            nc.sync.dma_start(out=iota16[:], in_=iota16_d.ap())
            nc.sync.dma_start(out=w1c0[:], in_=w1c0_d.ap())
            nc.sync.dma_start(out=w1c1[:], in_=w1c1_d.ap())
            nc.sync.dma_start(out=w2[:], in_=w2_d.ap())
            nc.sync.dma_start(out=b1[:], in_=b1_d.ap())
            nc.sync.dma_start(out=s16[:], in_=s16_d.ap())

            loop_cm = (tc.For_i(0, reps, 1) if reps > 1
                       else contextlib.nullcontext())
            with loop_cm:
              for w in range(n_win):
                hb_sb = hbp.tile([128, t_w, HB_W], dt.bfloat16, tag="hb")
                th = t_w // 2
                nc.gpsimd.dma_start(out=hb_sb[:, :th, :],
                                    in_=hb_wv[w][:, :th * HB_W])
                nc.gpsimd.dma_start(out=hb_sb[:, th:, :],
                                    in_=hb_wv[w][:, th * HB_W:])
                ht0_sb = htp.tile([D, B], dt.bfloat16, tag="h0")
                ht1_sb = htp.tile([D, B], dt.bfloat16, tag="h1")
                bh = B // 2
                nc.sync.dma_start(out=ht0_sb[:, :bh],
                                  in_=ht0_d.ap()[:, w * B:w * B + bh])
                nc.sync.dma_start(out=ht0_sb[:, bh:],
                                  in_=ht0_d.ap()[:, w * B + bh:(w + 1) * B])
                nc.sync.dma_start(out=ht1_sb[:, :bh],
                                  in_=ht1_d.ap()[:, w * B:w * B + bh])
                nc.sync.dma_start(out=ht1_sb[:, bh:],
                                  in_=ht1_d.ap()[:, w * B + bh:(w + 1) * B])

                psw = ps_w.tile([WSIZE * H, F_IN + 1], dt.float32)
                for (j0, ns) in macros:
                    # score path in halves of <=512 nodes
                    sco = ps_sco.tile([128, MACRO * H], dt.float32,
                                      tag="sco")
                    n_half = (ns * 128 + 511) // 512
                    for hh in range(n_half):
                        f0 = j0 * 128 + hh * 512
                        fw = min(512, (j0 + ns) * 128 - f0)
                        t1 = ps_mm.tile([D, 512], dt.float32, tag="t1")
                        nc.tensor.matmul(t1[:, :fw], w1c0[:],
                                         ht0_sb[:, f0:f0 + fw],
                                         start=True, stop=False)
                        nc.tensor.matmul(t1[:, :fw], w1c1[:],
                                         ht1_sb[:, f0:f0 + fw],
                                         start=False, stop=True)
                        a1 = actp.tile([D, 512], dt.bfloat16, tag="a1")
                        nc.scalar.activation(a1[:, :fw], t1[:, :fw],
                                             AF.Tanh, bias=b1[:])
                        for jj in range(fw // 128):
                            j = hh * 4 + jj     # slot within macro
                            nc.tensor.matmul(
                                sco[:, j * H:(j + 1) * H],
                                a1[:, jj * 128:(jj + 1) * 128], w2[:],
                                start=True, stop=True)
                    e_sb = epool.tile([128, MACRO * H], dt.bfloat16,
                                      tag="e")
                    nc.scalar.activation(e_sb[:, :ns * H], sco[:, :ns * H],
                                         AF.Exp)
                    msk = epool.tile([128, MACRO * WSIZE], dt.bfloat16,
                                     tag="M")
                    g_b = hb_sb[:, j0:j0 + ns,
                                F_IN + 1:F_IN + 2].broadcast_to(
                                    (128, ns, WSIZE))
                    i_b = iota16[:].unsqueeze(1).broadcast_to(
                        (128, ns, WSIZE))
                    nc.vector.tensor_tensor(
                        msk[:, :ns * WSIZE].rearrange(
                            "p (j g) -> p j g", g=WSIZE),
                        g_b, i_b, mybir.AluOpType.is_equal)
                    em = epool.tile([128, MACRO * 128], dt.bfloat16,
                                    tag="E")
                    e_b = e_sb[:, :ns * H].rearrange(
                        "p (j k) -> p j k", k=H).unsqueeze(2).broadcast_to(
                            (128, ns, WSIZE, H))
                    m_b = msk[:, :ns * WSIZE].rearrange(
                        "p (j g) -> p j g", g=WSIZE).unsqueeze(3).broadcast_to(
                            (128, ns, WSIZE, H))
                    nc.vector.tensor_mul(
                        em[:, :ns * 128].rearrange(
                            "p (j g k) -> p j g k", g=WSIZE, k=H),
                        m_b, e_b)
                    for jj in range(ns):
                        j = j0 + jj
                        nc.tensor.matmul(
                            psw[:],
                            em[:, jj * 128:(jj + 1) * 128],
                            hb_sb[:, j, :F_IN + 1],
                            start=(j == 0), stop=(j == t_w - 1))

                # drain window
                dcl = drainp.tile([WSIZE * H, 1], dt.float32, tag="dcl")
                nc.vector.tensor_scalar_max(dcl[:], psw[:, F_IN:F_IN + 1],
                                            1e-30)
                rc = drainp.tile([WSIZE * H, 1], dt.float32, tag="rc")
                nc.vector.reciprocal(rc[:], dcl[:])
                ssc = drainp.tile([WSIZE * H, F_IN], dt.bfloat16, tag="ssc")
                nc.vector.tensor_scalar_mul(ssc[:], psw[:, :F_IN], rc[:])
                outp = ps_out.tile([WSIZE, F_IN], dt.float32)
                nc.tensor.matmul(outp[:], s16[:], ssc[:], start=True,
                                 stop=True)
                out_sb = drainp.tile([WSIZE, F_IN], dt.float32, tag="osb")
                nc.vector.tensor_copy(out_sb[:], outp[:])
                nc.scalar.dma_start(
                    out=out_d.ap()[w * WSIZE:(w + 1) * WSIZE, :],
                    in_=out_sb[:])

    nc.compile()
    return nc


# ---------------------------------------------------------------- jit runner
class _Runner:
    """Persistent sharded jit wrapper around the compiled Bass program.

    Mirrors bass2jax.run_bass_via_pjrt's multi-core path, but keeps the
    jitted callable and device-resident inputs so repeated executions (for
    timing) skip retrace/recompile/re-transfer.
    """

    def __init__(self, nc):
        import jax
        import concourse.mybir as mybir
        from concourse import bass2jax
        from jax.experimental.shard_map import shard_map
        from jax.sharding import Mesh, PartitionSpec

        bass2jax.install_neuronx_cc_hook()
        self.jax = jax
        part_name = (nc.partition_id_tensor.name
                     if nc.partition_id_tensor else None)
        in_names, out_names, out_avals, zero_outs = [], [], [], []
        for alloc in nc.m.functions[0].allocations:
            if not isinstance(alloc, mybir.MemoryLocationSet):
                continue
            name = alloc.memorylocations[0].name
            if alloc.kind == "ExternalInput":
                if name == part_name:
                    continue
                in_names.append(name)
            elif alloc.kind == "ExternalOutput":
                out_names.append(name)
                shape = tuple(alloc.tensor_shape)
                dtype = mybir.dt.np(alloc.dtype)
                out_avals.append(jax.core.ShapedArray(shape, dtype))
                zero_outs.append(np.zeros(shape, dtype))
        n_params = len(in_names)
        self.in_names = list(in_names)
        self.out_names = out_names
        self.out_avals = out_avals
        self.zero_outs = zero_outs

        bind_names = list(in_names) + list(out_names)
        if part_name is not None:
            bind_names.append(part_name)

        def _body(*args):
            operands = list(args)
            if part_name is not None:
                operands.append(bass2jax.partition_id_tensor())
            outs = bass2jax._bass_exec_p.bind(
                *operands,
                out_avals=tuple(out_avals),
                in_names=tuple(bind_names),
                out_names=tuple(out_names),
                lowering_input_output_aliases=(),
                sim_require_finite=True,
                sim_require_nnan=True,
                nc=nc,
            )
            return tuple(outs)

        devices = jax.devices()[:N_CORES]
        self.mesh = Mesh(np.asarray(devices), ("core",))
        self.pspec = PartitionSpec("core")
        in_specs = (self.pspec,) * (n_params + len(out_names))
        out_specs = (self.pspec,) * len(out_names)
        donate = tuple(range(n_params, n_params + len(out_names)))
        self.sharded = jax.jit(
            shard_map(_body, mesh=self.mesh, in_specs=in_specs,
                      out_specs=out_specs, check_rep=False),
            donate_argnums=donate, keep_unused=True)

    def put_inputs(self, in_maps):
        import jax
        from jax.sharding import NamedSharding
        sh = NamedSharding(self.mesh, self.pspec)
        self.dev_in = [
            jax.device_put(
                np.concatenate([np.asarray(m[name]) for m in in_maps],
                               axis=0), sh)
            for name in self.in_names]

    def run(self, block=True):
        import jax
        from jax.sharding import NamedSharding
        sh = NamedSharding(self.mesh, self.pspec)
        zeros = [jax.device_put(
            np.zeros((N_CORES * z.shape[0], *z.shape[1:]), z.dtype), sh)
            for z in self.zero_outs]
        out = self.sharded(*self.dev_in, *zeros)
        if block:
            jax.block_until_ready(out)
        return out

    def timed_burst(self, n):
        """Dispatch n executions async, block at the end; return wall s."""
        import jax
        import time as _t
        t0 = _t.perf_counter()
        out = None
        for _ in range(n):
            out = self.run(block=False)
        jax.block_until_ready(out)
        return _t.perf_counter() - t0

    def results(self, out_arrs):
        return [
            {name: np.asarray(out_arrs[i]).reshape(
                N_CORES, *self.out_avals[i].shape)[c]
             for i, name in enumerate(self.out_names)}
            for c in range(N_CORES)]


_RUNNER_CACHE = {}


# ------------------------------------------------------------------- kernel()
def kernel(h, segment_ids, W1, b1, W2, b2, num_graphs):
    h = np.asarray(h)
    segment_ids = np.asarray(segment_ids)
    G = int(num_graphs)

    hb_all, ht0_all, ht1_all, meta = _preprocess(h, segment_ids, G)
    consts = _const_inputs(W1, b1, W2, b2)

    key = (meta["n_win"], meta["t_w"], meta["npad"])
    if key not in _RUNNER_CACHE:
        nc = _build_program(meta["n_win"], meta["t_w"], meta["npad"],
                            N_CORES)
        _RUNNER_CACHE[key] = _Runner(nc)
    runner = _RUNNER_CACHE[key]

    in_maps = []
    for c in range(N_CORES):
        m = {"hb": hb_all[c], "ht0": ht0_all[c], "ht1": ht1_all[c]}
        m.update(consts)
        in_maps.append(m)
    runner.put_inputs(in_maps)

    out_arrs = runner.run()   # first call compiles NEFF
    reps = int(os.environ.get("KERNEL_TIME_REPS", "0"))
    if reps:
        n_lo, n_hi = 2, 2 + reps
        t_lo = min(runner.timed_burst(n_lo) for _ in range(3))
        t_hi = min(runner.timed_burst(n_hi) for _ in range(3))
        slope = (t_hi - t_lo) / (n_hi - n_lo)
        print(f"burst timing: n={n_lo}: {t_lo*1e3:.2f} ms, "
              f"n={n_hi}: {t_hi*1e3:.2f} ms")
        print(f"HW exec time: {int(slope * 1e9)} ns")

    res = runner.results(out_arrs)
    out = np.zeros((G, F_IN), dtype=np.float32)
    r2g = meta["row2graph"]
    for c in range(N_CORES):
        valid = r2g[c] >= 0
        out[r2g[c][valid]] = res[c]["out"][valid]
    return out
